# revision 24
# baseline (speedup 1.0000x reference)
"""Trainium2 Bass kernel for nn_AttnDBGNNLayer (8-core SPMD).

kernel(**inputs) takes the FULL inputs (as produced by setup_inputs) and
returns the FULL output (new_A, new_B), distributing across 8 NeuronCores.

Design:
- q-rows of both attentions sharded 8-way (1024 rows/core); K/V computed
  replicated from a feature-major x0^T; single-pass unnormalized softmax
  (scores are tiny, no max subtraction); out-projection folded into V
  (Wvo = Wout @ Wv); row-sum via DVE accumulation + ones-matmul; normalize
  via PE-transpose + per-partition scale.
- per-graph gather tables hold h @ wl^T (lin_l folded), AllGathered in bf16.
- message aggregation as dense count-matrix matmuls: out^T += tab_g^T @ C_g
  with C_g the per-core [8192 src, 1024 dst] edge-count matrix in fp8
  (counts are small ints, exact); lin_r / biases / degree corrections are
  folded in as K=1 matmuls into the same PSUM accumulation.
- outputs are produced feature-major and transposed on the host.
"""
import sys

if "/opt/trn_rl_repo" not in sys.path:
    sys.path.insert(0, "/opt/trn_rl_repo")

import numpy as np
import ml_dtypes

import concourse.bacc as bacc
import concourse.tile as tile
import concourse.mybir as mybir
from concourse import bass_utils

BF16 = ml_dtypes.bfloat16
FP8 = ml_dtypes.float8_e4m3

N = 8192          # nodes per type
D = 128           # feature dim
NCORES = 8
R = N // NCORES   # rows (q / dst nodes) per core = 1024
QG = 512          # q-group width
KB = N // 128     # number of 128-wide k blocks = 64
NBLK = R // 128   # dst blocks per core = 8
SCALE = 1.0 / np.sqrt(np.float32(D))

F32 = mybir.dt.float32
BF = mybir.dt.bfloat16
F8 = mybir.dt.float8e4

G = ("AB", "BA", "AA")
SRC_T = {"AB": "A", "BA": "B", "AA": "A"}
GRAPHS_OF = {"A": ("BA", "AA"), "B": ("AB",)}   # inbound graphs per output
TABLES_OF = {"A": ("AB", "AA"), "B": ("BA",)}   # tables whose src is t

_PROG_CACHE = {}


def build_program(dbg=False, stage=3):
    nc = bacc.Bacc("TRN2", target_bir_lowering=False, debug=False,
                   num_devices=NCORES)

    # ---- I/O (identical shapes on all cores; per-core data)
    x0t = {t: nc.dram_tensor(f"x0t_{t}", [128, N], BF, kind="ExternalInput")
           for t in "AB"}
    x0q = {t: nc.dram_tensor(f"x0q_{t}", [128, R], BF, kind="ExternalInput")
           for t in "AB"}
    wqT = {t: nc.dram_tensor(f"wqT_{t}", [128, 128], BF, kind="ExternalInput")
           for t in "AB"}
    wkT = {t: nc.dram_tensor(f"wkT_{t}", [128, 128], BF, kind="ExternalInput")
           for t in "AB"}
    wvoT = {t: nc.dram_tensor(f"wvoT_{t}", [128, 128], BF, kind="ExternalInput")
            for t in "AB"}
    bqs = {t: nc.dram_tensor(f"bqs_{t}", [128, 1], F32, kind="ExternalInput")
           for t in "AB"}
    bk = {t: nc.dram_tensor(f"bk_{t}", [128, 1], F32, kind="ExternalInput")
          for t in "AB"}
    wlT = {g: nc.dram_tensor(f"wlT_{g}", [128, 128], BF, kind="ExternalInput")
           for g in G}
    wrT = {t: nc.dram_tensor(f"wrT_{t}", [128, 128], BF, kind="ExternalInput")
           for t in "AB"}
    c0 = {t: nc.dram_tensor(f"c0_{t}", [1, 128], F32, kind="ExternalInput")
          for t in "AB"}
    c1 = {g: nc.dram_tensor(f"c1_{g}", [1, 128], F32, kind="ExternalInput")
          for g in G}
    deg = {g: nc.dram_tensor(f"deg_{g}", [1, R], F32, kind="ExternalInput")
           for g in G}
    ct = {g: nc.dram_tensor(f"ct_{g}", [1024, 8 * R], F8, kind="ExternalInput")
          for g in G}
    # feature-major outputs; host transposes
    out_d = {t: nc.dram_tensor(f"out_{t}", [128, R], F32,
                               kind="ExternalOutput") for t in "AB"}
    dbg_d = {}
    if dbg:
        for t in "AB":
            dbg_d[f"ht_{t}"] = nc.dram_tensor(f"dbg_ht_{t}", [128, R], BF,
                                              kind="ExternalOutput")
        for g in G:
            dbg_d[f"tab_{g}"] = nc.dram_tensor(f"dbg_tab_{g}", [N, 128], BF,
                                               kind="ExternalOutput")

    # internal DRAM for collectives
    tab_loc = {g: nc.dram_tensor(f"tabloc_{g}", [R, 128], BF) for g in G}
    tab = {g: nc.dram_tensor(f"tab_{g}", [N, 128], BF, addr_space="Shared")
           for g in G}

    with tile.TileContext(nc) as tc:
        with (
            tc.tile_pool(name="const", bufs=1) as cp,
            tc.tile_pool(name="big", bufs=1) as bp,
            tc.tile_pool(name="pt", bufs=3) as ptp,
            tc.tile_pool(name="ctp", bufs=6) as ctp,
            tc.tile_pool(name="ps_s", bufs=2, space="PSUM") as ps_s,
            tc.tile_pool(name="ps_u", bufs=2, space="PSUM") as ps_u,
            tc.tile_pool(name="ps_sm", bufs=2, space="PSUM") as ps_sm,
        ):
            # ---------------- constants into SBUF
            def load_const(dram, shape, dt):
                t = cp.tile(shape, dt, tag=dram.name)
                nc.sync.dma_start(out=t[:], in_=dram[:])
                return t

            wqT_s = {t: load_const(wqT[t], [128, 128], BF) for t in "AB"}
            wkT_s = {t: load_const(wkT[t], [128, 128], BF) for t in "AB"}
            wvoT_s = {t: load_const(wvoT[t], [128, 128], BF) for t in "AB"}
            bqs_s = {t: load_const(bqs[t], [128, 1], F32) for t in "AB"}
            bk_s = {t: load_const(bk[t], [128, 1], F32) for t in "AB"}
            wlT_s = {g: load_const(wlT[g], [128, 128], BF) for g in G}
            wrT_s = {t: load_const(wrT[t], [128, 128], BF) for t in "AB"}
            c0_s = {t: load_const(c0[t], [1, 128], F32) for t in "AB"}
            c1_s = {g: load_const(c1[g], [1, 128], F32) for g in G}
            deg_s = {g: load_const(deg[g], [1, R], F32) for g in G}

            ident = cp.tile([128, 128], BF, tag="ident")
            from concourse.masks import make_identity
            make_identity(nc, ident[:])
            ones_col = cp.tile([128, 1], BF, tag="ones_col")
            nc.vector.memset(ones_col[:], 1.0)
            ones_row = cp.tile([1, 512], F32, tag="ones_row")
            nc.vector.memset(ones_row[:], 1.0)

            ht = {t: bp.tile([128, R], BF, tag=f"ht_{t}", name=f"ht_{t}")
                  for t in "AB"}

            # ---------------- QKV for both types, then interleaved attention
            kt = {}
            vt = {}
            qt = {}
            for t in "AB":
                x0_s = bp.tile([128, N], BF, tag="x0t", name=f"x0_{t}")
                nc.sync.dma_start(out=x0_s[:], in_=x0t[t][:])
                x0q_s = bp.tile([128, R], BF, tag="x0q", name=f"x0q_{t}")
                nc.sync.dma_start(out=x0q_s[:], in_=x0q[t][:])

                kt[t] = bp.tile([128, N], BF, tag=f"kt_{t}", name=f"kt_{t}")
                vt[t] = bp.tile([128, N], BF, tag=f"vt_{t}", name=f"vt_{t}")
                qt[t] = bp.tile([128, R], BF, tag=f"qt_{t}", name=f"qt_{t}")

                # Kt = Wk @ x0T (+bk), feature-major
                for j in range(N // 512):
                    ps = ps_s.tile([128, 1024], F32, tag="sc")
                    nc.tensor.matmul(ps[:, :512], lhsT=wkT_s[t][:],
                                     rhs=x0_s[:, j * 512:(j + 1) * 512],
                                     start=True, stop=True)
                    nc.scalar.activation(kt[t][:, j * 512:(j + 1) * 512],
                                         ps[:, :512],
                                         mybir.ActivationFunctionType.Identity,
                                         bias=bk_s[t][:, :])
                # Qt = s*(Wq @ x0T_own + bq)
                for j in range(R // 512):
                    ps = ps_s.tile([128, 1024], F32, tag="sc")
                    nc.tensor.matmul(ps[:, :512], lhsT=wqT_s[t][:],
                                     rhs=x0q_s[:, j * 512:(j + 1) * 512],
                                     start=True, stop=True)
                    nc.scalar.activation(qt[t][:, j * 512:(j + 1) * 512],
                                         ps[:, :512],
                                         mybir.ActivationFunctionType.Identity,
                                         bias=bqs_s[t][:, :], scale=float(SCALE))
                # V~ = x0 @ (Wout Wv)^T, node-major blocks (4 per psum tile)
                for vg in range(KB // 4):
                    ps = ps_u.tile([128, 512], F32, tag="ut")
                    for i in range(4):
                        nb = vg * 4 + i
                        nc.tensor.matmul(ps[:, i * 128:(i + 1) * 128],
                                         lhsT=x0_s[:, nb * 128:(nb + 1) * 128],
                                         rhs=wvoT_s[t][:], start=True, stop=True)
                    nc.vector.tensor_copy(vt[t][:, vg * 512:(vg + 1) * 512],
                                          ps[:])

            # attention: A and B q-groups interleaved (keeps PE saturated)
            for qg in range(R // QG):
                q_sl = slice(qg * QG, (qg + 1) * QG)
                ut_ps = {}
                racc0 = {}
                racc1 = {}
                for t in "AB":
                    ut_ps[t] = ps_u.tile([128, QG], F32, tag="ut",
                                         name=f"utps_{t}_{qg}")
                    racc0[t] = bp.tile([128, QG], BF, tag=f"racc0_{t}",
                                       name=f"racc0_{t}_{qg}")
                    racc1[t] = bp.tile([128, QG], BF, tag=f"racc1_{t}",
                                       name=f"racc1_{t}_{qg}")
                    nc.vector.memset(racc0[t][:], 0.0)
                    nc.vector.memset(racc1[t][:], 0.0)
                for pr in range(KB // 2):
                    kb0 = 2 * pr
                    for t in "AB":
                        sc = ps_s.tile([128, 1024], F32, tag="sc",
                                       name=f"sc_{t}_{pr}")
                        nc.tensor.matmul(sc[:, :512],
                                         lhsT=kt[t][:, kb0 * 128:(kb0 + 1) * 128],
                                         rhs=qt[t][:, q_sl],
                                         start=True, stop=True)
                        nc.tensor.matmul(sc[:, 512:],
                                         lhsT=kt[t][:, (kb0 + 1) * 128:(kb0 + 2) * 128],
                                         rhs=qt[t][:, q_sl],
                                         start=True, stop=True)
                        pt = ptp.tile([128, 1024], BF, tag="pt",
                                      name=f"pt_{t}_{pr}")
                        nc.scalar.activation(pt[:], sc[:],
                                             mybir.ActivationFunctionType.Exp)
                        nc.tensor.matmul(ut_ps[t][:],
                                         lhsT=vt[t][:, kb0 * 128:(kb0 + 1) * 128],
                                         rhs=pt[:, :512],
                                         start=(pr == 0), stop=False)
                        nc.tensor.matmul(ut_ps[t][:],
                                         lhsT=vt[t][:, (kb0 + 1) * 128:(kb0 + 2) * 128],
                                         rhs=pt[:, 512:],
                                         start=False, stop=(pr == KB // 2 - 1))
                        nc.vector.tensor_add(racc0[t][:], racc0[t][:],
                                             pt[:, :512])
                        nc.vector.tensor_add(racc1[t][:], racc1[t][:],
                                             pt[:, 512:])

                for t in "AB":
                    ut_sb = bp.tile([128, QG], BF, tag=f"ut_sb_{t}",
                                    name=f"ut_sb_{t}_{qg}")
                    nc.vector.tensor_copy(ut_sb[:], ut_ps[t][:])
                    for sub in range(QG // 128):
                        s_sl = slice(sub * 128, (sub + 1) * 128)
                        rp = ps_sm.tile([128, 512], F32, tag="sm", name="rp")
                        nc.tensor.matmul(rp[:, :1], lhsT=racc0[t][:, s_sl],
                                         rhs=ones_col[:], start=True,
                                         stop=False)
                        nc.tensor.matmul(rp[:, :1], lhsT=racc1[t][:, s_sl],
                                         rhs=ones_col[:], start=False,
                                         stop=True)
                        rinv = bp.tile([128, 1], F32, tag="rinv")
                        nc.vector.reciprocal(rinv[:], rp[:, :1])
                        tp = ps_sm.tile([128, 512], BF, tag="sm", name="tp")
                        nc.tensor.transpose(tp[:, :128], ut_sb[:, s_sl],
                                            ident[:])
                        hn = bp.tile([128, 128], BF, tag="hn")
                        nc.scalar.activation(hn[:], tp[:, :128],
                                             mybir.ActivationFunctionType.Copy,
                                             scale=rinv[:, :])
                        tp2 = ps_sm.tile([128, 512], BF, tag="sm", name="tp2")
                        nc.tensor.transpose(tp2[:, :128], hn[:], ident[:])
                        nc.vector.tensor_copy(
                            ht[t][:, qg * QG + sub * 128:
                                  qg * QG + (sub + 1) * 128],
                            tp2[:, :128])

            # tables: tab_loc_g = (h_src @ wl_g^T) node-major, then AllGather
            for t in "AB":
                for g in TABLES_OF[t]:
                    tsb = bp.tile([128, NBLK * 128], BF, tag="tsb",
                                  name=f"tsb_{g}")
                    for nb in range(NBLK):
                        ps = ps_sm.tile([128, 512], F32, tag="sm", name="tps")
                        nc.tensor.matmul(ps[:, :128],
                                         lhsT=ht[t][:, nb * 128:(nb + 1) * 128],
                                         rhs=wlT_s[g][:], start=True, stop=True)
                        nc.vector.tensor_copy(tsb[:, nb * 128:(nb + 1) * 128],
                                              ps[:, :128])
                    for nb in range(NBLK):
                        nc.sync.dma_start(
                            out=tab_loc[g][nb * 128:(nb + 1) * 128, :],
                            in_=tsb[:, nb * 128:(nb + 1) * 128])
                    nc.gpsimd.collective_compute(
                        "AllGather", mybir.AluOpType.bypass,
                        replica_groups=[list(range(NCORES))],
                        ins=[tab_loc[g].ap()], outs=[tab[g].ap()])

            if dbg:
                for t in "AB":
                    nc.sync.dma_start(out=dbg_d[f"ht_{t}"][:], in_=ht[t][:])
                for g in G:
                    nc.sync.dma_start(out=dbg_d[f"tab_{g}"][:], in_=tab[g][:])

            # ---------------- phase 2: dense count-matrix aggregation
            # out^T[d, dst] = sum_g tab_g^T @ C_g + wr@ht + c0*1 + c1*deg
            for t in ("BA" if stage >= 2 else ""):
                po = []
                for h in range(2):
                    po_t = ps_u.tile([128, 512], F32, tag="ut",
                                     name=f"po_{t}_{h}")
                    po.append(po_t)
                first = [True, True]
                for g in GRAPHS_OF[t]:
                    # table node-major into SBUF: [s within block, blk*128+d]
                    tabsb = bp.tile([128, KB * 128], BF, tag="x0t",
                                    name=f"tabsb_{g}")
                    nc.sync.dma_start(
                        out=tabsb[:].rearrange("s (b d) -> s b d", d=128),
                        in_=tab[g][:].rearrange("(b s) d -> s b d", s=128))
                    for scg in range(8):   # groups of 8 s-blocks
                        ct_t = ctp.tile([128, 8 * R], F8, tag="ct",
                                        name=f"ct_{g}_{scg}")
                        nc.sync.dma_start(
                            out=ct_t[:],
                            in_=ct[g][scg * 128:(scg + 1) * 128, :])
                        for sb in range(8):
                            lt = tabsb[:, (scg * 8 + sb) * 128:
                                       (scg * 8 + sb + 1) * 128]
                            for h in range(2):
                                nc.tensor.matmul(
                                    po[h][:],
                                    lhsT=lt,
                                    rhs=ct_t[:, sb * R + h * 512:
                                             sb * R + (h + 1) * 512],
                                    start=first[h], stop=False)
                                first[h] = False
                    # degree correction: out^T += c1 (x) deg
                    for h in range(2):
                        nc.tensor.matmul(po[h][:], lhsT=c1_s[g][:],
                                         rhs=deg_s[g][:, h * 512:(h + 1) * 512],
                                         start=False, stop=False)
                # lin_r:  out^T += wr @ ht
                for h in range(2):
                    nc.tensor.matmul(po[h][:], lhsT=wrT_s[t][:],
                                     rhs=ht[t][:, h * 512:(h + 1) * 512],
                                     start=False, stop=False)
                    # constant c0 per feature
                    nc.tensor.matmul(po[h][:], lhsT=c0_s[t][:],
                                     rhs=ones_row[:], start=False, stop=True)
                    osb = bp.tile([128, 512], F32, tag="osb",
                                  name=f"osb_{t}_{h}")
                    nc.vector.tensor_copy(osb[:], po[h][:])
                    nc.sync.dma_start(out=out_d[t][:, h * 512:(h + 1) * 512],
                                      in_=osb[:])

    nc.compile()
    return nc


# ---------------------------------------------------------------- host prep

def _prep(inputs, dbg=False):
    ins = {k: np.asarray(v) for k, v in inputs.items()}

    def bf(x):
        return np.ascontiguousarray(np.asarray(x, np.float32)).astype(BF16)

    com = {}
    for t in "AB":
        iw = ins[f"inW_{t}"].astype(np.float32)
        ib = ins[f"inB_{t}"].astype(np.float32)
        ow = ins[f"outW_{t}"].astype(np.float32)
        ob = ins[f"outB_{t}"].astype(np.float32)
        com[f"wqT_{t}"] = bf(iw[0:128].T)
        com[f"wkT_{t}"] = bf(iw[128:256].T)
        wvo = ow @ iw[256:384]
        com[f"wvoT_{t}"] = bf(wvo.T)
        com[f"bqs_{t}"] = (ib[0:128] * SCALE).reshape(128, 1).astype(np.float32)
        com[f"bk_{t}"] = ib[128:256].reshape(128, 1).astype(np.float32)
        com[f"bout_eff_{t}"] = ow @ ib[256:384] + ob
    for g in G:
        com[f"wlT_{g}"] = bf(ins[f"wl_{g}"].T)
        com[f"c1_{g}"] = (ins[f"wl_{g}"].astype(np.float32)
                          @ com[f"bout_eff_{SRC_T[g]}"]).reshape(1, 128)
    com["wrT_B"] = bf(ins["wr_AB"].T)
    com["wrT_A"] = bf((ins["wr_BA"] + ins["wr_AA"]).T)
    com["c0_B"] = (ins["bl_AB"].astype(np.float32)
                   + ins["wr_AB"].astype(np.float32) @ com["bout_eff_B"]
                   ).reshape(1, 128)
    com["c0_A"] = (ins["bl_BA"].astype(np.float32)
                   + ins["bl_AA"].astype(np.float32)
                   + (ins["wr_BA"] + ins["wr_AA"]).astype(np.float32)
                   @ com["bout_eff_A"]).reshape(1, 128)

    x0T = {t: np.ascontiguousarray(
        ins[f"x_{t}"][:, 0, :].astype(np.float32).T).astype(BF16)
        for t in "AB"}

    # per-core count matrices [N src, R dst] fp8 + degree vectors
    cts = {}
    degs = {}
    for g in G:
        src = np.asarray(ins[f"ei_{g}"][0], np.int64)
        dst = np.asarray(ins[f"ei_{g}"][1], np.int64)
        per_core = []
        dgs = []
        for c in range(NCORES):
            sel = (dst >> 10) == c
            s_c = src[sel]
            d_c = dst[sel] - c * R
            cmat = np.zeros((N, R), np.float32)
            np.add.at(cmat, (s_c, d_c), 1.0)
            # swizzle to [8 scg, 128 s-in-block, 8 sb, R] -> [1024, 8R]
            swz = np.ascontiguousarray(
                cmat.reshape(8, 8, 128, R).transpose(0, 2, 1, 3)
                .reshape(1024, 8 * R))
            per_core.append(swz.astype(FP8))
            dgs.append(np.bincount(d_c, minlength=R).astype(np.float32)
                       .reshape(1, R))
        cts[g] = per_core
        degs[g] = dgs

    in_maps = []
    for c in range(NCORES):
        m = {}
        for t in "AB":
            m[f"x0t_{t}"] = x0T[t]
            m[f"x0q_{t}"] = np.ascontiguousarray(x0T[t][:, c * R:(c + 1) * R])
            for k in ("wqT", "wkT", "wvoT", "bqs", "bk", "wrT", "c0"):
                m[f"{k}_{t}"] = com[f"{k}_{t}"]
        for g in G:
            m[f"wlT_{g}"] = com[f"wlT_{g}"]
            m[f"c1_{g}"] = com[f"c1_{g}"].astype(np.float32)
            m[f"deg_{g}"] = degs[g][c]
            m[f"ct_{g}"] = cts[g][c]
        in_maps.append(m)
    return in_maps


def kernel(**inputs):
    in_maps = _prep(inputs)
    if "prog" not in _PROG_CACHE:
        _PROG_CACHE["prog"] = build_program()
    nc = _PROG_CACHE["prog"]
    res = bass_utils.run_bass_kernel_spmd(
        nc, in_maps, core_ids=list(range(NCORES)))
    x_A = np.asarray(inputs["x_A"], np.float32)
    x_B = np.asarray(inputs["x_B"], np.float32)
    new_A = x_A.copy()
    new_B = x_B.copy()
    for c in range(NCORES):
        new_A[c * R:(c + 1) * R, 0, :] = res.results[c]["out_A"].T
        new_B[c * R:(c + 1) * R, 0, :] = res.results[c]["out_B"].T
    return new_A, new_B


# revision 27
# speedup vs baseline: 1.0841x; 1.0841x over previous
"""Trainium2 Bass kernel for nn_AttnDBGNNLayer (8-core SPMD).

kernel(**inputs) takes the FULL inputs (as produced by setup_inputs) and
returns the FULL output (new_A, new_B), distributing across 8 NeuronCores.

Design:
- q-rows of both attentions sharded 8-way (1024 rows/core); K/V computed
  replicated from a feature-major x0^T; A and B q-groups interleaved so the
  TensorEngine always has independent work; single-pass unnormalized softmax
  (scores are tiny; no max subtraction); out-projection folded into V
  (Wvo = Wout @ Wv); softmax row-sum accumulated on DVE+GpSimd, finished
  with a ones-matmul; normalize via PE-transpose + per-partition scale.
- per-graph tables hold h @ wl^T (lin_l folded); the three tables are
  concatenated and AllGathered in two row-halves so the first collective
  hides under the second attention q-group.
- message aggregation as dense count-matrix matmuls: out^T += tab_g^T @ C_g
  with C_g the per-core [8192 src, 1024 dst] edge-count matrix in fp8
  (counts are small ints -> exact); C rows are host-permuted to match the
  AllGather row order and host-swizzled for contiguous streaming.
  lin_r / biases / degree corrections fold in as K=1 matmuls into the same
  PSUM accumulation group.
- outputs are produced feature-major and transposed on the host.
"""
import sys

if "/opt/trn_rl_repo" not in sys.path:
    sys.path.insert(0, "/opt/trn_rl_repo")

import numpy as np
import ml_dtypes

import concourse.bacc as bacc
import concourse.tile as tile
import concourse.mybir as mybir
from concourse import bass_utils

BF16 = ml_dtypes.bfloat16
FP8 = ml_dtypes.float8_e4m3

N = 8192
D = 128
NCORES = 8
R = N // NCORES       # 1024 rows per core
QG = 512              # q-group width
KB = N // 128         # 64 k-blocks
NBLK = R // 128       # 8 dst blocks per core
SCALE = 1.0 / np.sqrt(np.float32(D))

F32 = mybir.dt.float32
BF = mybir.dt.bfloat16
F8 = mybir.dt.float8e4

G = ("AB", "BA", "AA")
GI = {g: i for i, g in enumerate(G)}
SRC_T = {"AB": "A", "BA": "B", "AA": "A"}
GRAPHS_OF = {"A": ("BA", "AA"), "B": ("AB",)}
TABLES_OF = {"A": ("AB", "AA"), "B": ("BA",)}

# bf16 weight blob layout: [128,128] slices
WB_ORDER = ["wqT_A", "wkT_A", "wvoT_A", "wqT_B", "wkT_B", "wvoT_B",
            "wlT_AB", "wlT_BA", "wlT_AA", "wrT_A", "wrT_B"]
# f32 col blob: [128, 4]
CB_ORDER = ["bqs_A", "bk_A", "bqs_B", "bk_B"]
# f32 row blob: [1, 128*5 + 1024*3]
RB_ORDER = ["c0_A", "c0_B", "c1_AB", "c1_BA", "c1_AA"]

_PROG_CACHE = {}


def build_program(dbg=False, stage=3):
    nc = bacc.Bacc("TRN2", target_bir_lowering=False, debug=False,
                   num_devices=NCORES)

    x0t = {t: nc.dram_tensor(f"x0t_{t}", [128, N], BF, kind="ExternalInput")
           for t in "AB"}
    x0q = {t: nc.dram_tensor(f"x0q_{t}", [128, R], BF, kind="ExternalInput")
           for t in "AB"}
    wblob = nc.dram_tensor("wblob", [128, 128 * len(WB_ORDER)], BF,
                           kind="ExternalInput")
    cblob = nc.dram_tensor("cblob", [128, len(CB_ORDER)], F32,
                           kind="ExternalInput")
    rblob = nc.dram_tensor("rblob", [1, 128 * 5 + R * 3], F32,
                           kind="ExternalInput")
    ct = {g: nc.dram_tensor(f"ct_{g}", [1024, 8 * R], F8,
                            kind="ExternalInput") for g in G}
    out_d = {t: nc.dram_tensor(f"out_{t}", [128, R], F32,
                               kind="ExternalOutput") for t in "AB"}
    dbg_d = {}
    if dbg:
        for t in "AB":
            dbg_d[f"ht_{t}"] = nc.dram_tensor(f"dbg_ht_{t}", [128, R], BF,
                                              kind="ExternalOutput")
        dbg_d["tab"] = nc.dram_tensor("dbg_tab", [N, 384], BF,
                                      kind="ExternalOutput")

    tab_loc = nc.dram_tensor("tab_loc", [R, 384], BF)
    tab = nc.dram_tensor("tab", [N, 384], BF, addr_space="Shared")

    with tile.TileContext(nc) as tc:
        with (
            tc.tile_pool(name="const", bufs=1) as cp,
            tc.tile_pool(name="big", bufs=1) as bp,
            tc.tile_pool(name="pt", bufs=4) as ptp,
            tc.tile_pool(name="ctp", bufs=7) as ctp,
            tc.tile_pool(name="ps_s", bufs=2, space="PSUM") as ps_s,
            tc.tile_pool(name="ps_u", bufs=2, space="PSUM") as ps_u,
            tc.tile_pool(name="ps_sm", bufs=2, space="PSUM") as ps_sm,
        ):
            # ---------------- inputs: x0 first, then const blobs
            x0_s = {}
            x0q_s = {}
            for t in "AB":
                x0_s[t] = bp.tile([128, N], BF, tag=f"x0t_{t}",
                                  name=f"x0_{t}")
                nc.sync.dma_start(out=x0_s[t][:], in_=x0t[t][:])
                x0q_s[t] = bp.tile([128, R], BF, tag=f"x0q_{t}",
                                   name=f"x0q_{t}")
                nc.sync.dma_start(out=x0q_s[t][:], in_=x0q[t][:])

            wb = cp.tile([128, 128 * len(WB_ORDER)], BF, tag="wb")
            nc.sync.dma_start(out=wb[:], in_=wblob[:])
            W = {k: wb[:, i * 128:(i + 1) * 128]
                 for i, k in enumerate(WB_ORDER)}
            cb = cp.tile([128, len(CB_ORDER)], F32, tag="cb")
            nc.sync.dma_start(out=cb[:], in_=cblob[:])
            C = {k: cb[:, i:i + 1] for i, k in enumerate(CB_ORDER)}
            rb = cp.tile([1, 128 * 5 + R * 3], F32, tag="rb")
            nc.sync.dma_start(out=rb[:], in_=rblob[:])
            RW = {k: rb[:, i * 128:(i + 1) * 128]
                  for i, k in enumerate(RB_ORDER)}
            DEG = {g: rb[:, 640 + GI[g] * R: 640 + (GI[g] + 1) * R] for g in G}

            ident = cp.tile([128, 128], BF, tag="ident")
            from concourse.masks import make_identity
            make_identity(nc, ident[:])
            ones_col = cp.tile([128, 1], BF, tag="ones_col")
            nc.vector.memset(ones_col[:], 1.0)
            ones_row = cp.tile([1, 512], F32, tag="ones_row")
            nc.vector.memset(ones_row[:], 1.0)

            ht = {t: bp.tile([128, R], BF, tag=f"ht_{t}", name=f"ht_{t}")
                  for t in "AB"}

            # ---------------- QKV for both types
            kt = {}
            vt = {}
            qt = {}
            for t in "AB":
                kt[t] = bp.tile([128, N], BF, tag=f"kt_{t}", name=f"kt_{t}")
                vt[t] = bp.tile([128, N], BF, tag=f"vt_{t}", name=f"vt_{t}")
                qt[t] = bp.tile([128, R], BF, tag=f"qt_{t}", name=f"qt_{t}")
                for j in range(N // 512):
                    ps = ps_s.tile([128, 1024], F32, tag="sc")
                    nc.tensor.matmul(ps[:, :512], lhsT=W[f"wkT_{t}"],
                                     rhs=x0_s[t][:, j * 512:(j + 1) * 512],
                                     start=True, stop=True)
                    nc.scalar.activation(kt[t][:, j * 512:(j + 1) * 512],
                                         ps[:, :512],
                                         mybir.ActivationFunctionType.Identity,
                                         bias=C[f"bk_{t}"])
                for j in range(R // 512):
                    ps = ps_s.tile([128, 1024], F32, tag="sc")
                    nc.tensor.matmul(ps[:, :512], lhsT=W[f"wqT_{t}"],
                                     rhs=x0q_s[t][:, j * 512:(j + 1) * 512],
                                     start=True, stop=True)
                    nc.scalar.activation(qt[t][:, j * 512:(j + 1) * 512],
                                         ps[:, :512],
                                         mybir.ActivationFunctionType.Identity,
                                         bias=C[f"bqs_{t}"],
                                         scale=float(SCALE))
                for vg in range(KB // 4):
                    ps = ps_u.tile([128, 512], F32, tag="ut")
                    for i in range(4):
                        nb = vg * 4 + i
                        nc.tensor.matmul(ps[:, i * 128:(i + 1) * 128],
                                         lhsT=x0_s[t][:, nb * 128:(nb + 1) * 128],
                                         rhs=W[f"wvoT_{t}"],
                                         start=True, stop=True)
                    nc.vector.tensor_copy(vt[t][:, vg * 512:(vg + 1) * 512],
                                          ps[:])

            # ---------------- attention, A/B interleaved; tables per half
            for qg in range(R // QG):
                q_sl = slice(qg * QG, (qg + 1) * QG)
                ut_ps = {}
                racc0 = {}
                racc1 = {}
                for t in "AB":
                    ut_ps[t] = ps_u.tile([128, QG], F32, tag="ut",
                                         name=f"utps_{t}_{qg}")
                    racc0[t] = bp.tile([128, QG], BF, tag=f"racc0_{t}",
                                       name=f"racc0_{t}_{qg}")
                    racc1[t] = bp.tile([128, QG], BF, tag=f"racc1_{t}",
                                       name=f"racc1_{t}_{qg}")
                    nc.vector.memset(racc0[t][:], 0.0)
                    nc.gpsimd.memset(racc1[t][:], 0.0)
                for pr in range(KB // 2):
                    kb0 = 2 * pr
                    for t in "AB":
                        sc = ps_s.tile([128, 1024], F32, tag="sc",
                                       name=f"sc_{t}_{pr}")
                        nc.tensor.matmul(sc[:, :512],
                                         lhsT=kt[t][:, kb0 * 128:(kb0 + 1) * 128],
                                         rhs=qt[t][:, q_sl],
                                         start=True, stop=True)
                        nc.tensor.matmul(sc[:, 512:],
                                         lhsT=kt[t][:, (kb0 + 1) * 128:(kb0 + 2) * 128],
                                         rhs=qt[t][:, q_sl],
                                         start=True, stop=True)
                        pt = ptp.tile([128, 1024], BF, tag="pt",
                                      name=f"pt_{t}_{pr}")
                        nc.scalar.activation(pt[:], sc[:],
                                             mybir.ActivationFunctionType.Exp)
                        nc.tensor.matmul(ut_ps[t][:],
                                         lhsT=vt[t][:, kb0 * 128:(kb0 + 1) * 128],
                                         rhs=pt[:, :512],
                                         start=(pr == 0), stop=False)
                        nc.tensor.matmul(ut_ps[t][:],
                                         lhsT=vt[t][:, (kb0 + 1) * 128:(kb0 + 2) * 128],
                                         rhs=pt[:, 512:],
                                         start=False, stop=(pr == KB // 2 - 1))
                        nc.vector.tensor_add(racc0[t][:], racc0[t][:],
                                             pt[:, :512])
                        nc.gpsimd.tensor_tensor(racc1[t][:], racc1[t][:],
                                                pt[:, 512:],
                                                op=mybir.AluOpType.add)

                # normalize + both orientations of h
                for t in "AB":
                    ut_sb = bp.tile([128, QG], BF, tag=f"ut_sb_{t}",
                                    name=f"ut_sb_{t}_{qg}")
                    nc.vector.tensor_copy(ut_sb[:], ut_ps[t][:])
                    for sub in range(QG // 128):
                        s_sl = slice(sub * 128, (sub + 1) * 128)
                        rp = ps_sm.tile([128, 512], F32, tag="sm", name="rp")
                        nc.tensor.matmul(rp[:, :1], lhsT=racc0[t][:, s_sl],
                                         rhs=ones_col[:], start=True,
                                         stop=False)
                        nc.tensor.matmul(rp[:, :1], lhsT=racc1[t][:, s_sl],
                                         rhs=ones_col[:], start=False,
                                         stop=True)
                        rinv = bp.tile([128, 1], F32, tag="rinv")
                        nc.vector.reciprocal(rinv[:], rp[:, :1])
                        tp = ps_sm.tile([128, 512], BF, tag="sm", name="tp")
                        nc.tensor.transpose(tp[:, :128], ut_sb[:, s_sl],
                                            ident[:])
                        hn = bp.tile([128, 128], BF, tag="hn")
                        nc.vector.tensor_scalar_mul(hn[:], tp[:, :128],
                                                    rinv[:, :])
                        tp2 = ps_sm.tile([128, 512], BF, tag="sm", name="tp2")
                        nc.tensor.transpose(tp2[:, :128], hn[:], ident[:])
                        nc.vector.tensor_copy(
                            ht[t][:, qg * QG + sub * 128:
                                  qg * QG + (sub + 1) * 128],
                            tp2[:, :128])

                # tables for this half: local rows qg*512..+512, all graphs
                for t in "AB":
                    for g in TABLES_OF[t]:
                        tsb = bp.tile([128, 4 * 128], BF, tag="tsb",
                                      name=f"tsb_{g}_{qg}")
                        for nb in range(4):
                            wblk = qg * 4 + nb
                            ps = ps_sm.tile([128, 512], F32, tag="sm",
                                            name="tps")
                            nc.tensor.matmul(
                                ps[:, :128],
                                lhsT=ht[t][:, wblk * 128:(wblk + 1) * 128],
                                rhs=W[f"wlT_{g}"], start=True, stop=True)
                            nc.vector.tensor_copy(
                                tsb[:, nb * 128:(nb + 1) * 128], ps[:, :128])
                        for nb in range(4):
                            wblk = qg * 4 + nb
                            nc.sync.dma_start(
                                out=tab_loc[wblk * 128:(wblk + 1) * 128,
                                            GI[g] * 128:(GI[g] + 1) * 128],
                                in_=tsb[:, nb * 128:(nb + 1) * 128])
                # half AllGather: rows [qg*512, qg*512+512) of each core
                nc.gpsimd.collective_compute(
                    "AllGather", mybir.AluOpType.bypass,
                    replica_groups=[list(range(NCORES))],
                    ins=[tab_loc[qg * 512:(qg + 1) * 512, :]],
                    outs=[tab[qg * 4096:(qg + 1) * 4096, :]])

            if dbg:
                for t in "AB":
                    nc.sync.dma_start(out=dbg_d[f"ht_{t}"][:], in_=ht[t][:])
                nc.sync.dma_start(out=dbg_d["tab"][:], in_=tab[:])

            # ---------------- phase 2: dense count-matrix aggregation
            # out^T[d, dst] = sum_g tab_g^T @ C_g + wr@ht + c0*1 + c1*deg
            for t in ("BA" if stage >= 2 else ""):
                po = []
                for h in range(2):
                    po_t = ps_u.tile([128, 512], F32, tag="ut",
                                     name=f"po_{t}_{h}")
                    po.append(po_t)
                first = [True, True]
                for g in GRAPHS_OF[t]:
                    tabsb = bp.tile([128, KB * 128], BF,
                                    tag=f"x0t_{'A' if GI[g] % 2 == 0 else 'B'}",
                                    name=f"tabsb_{g}")
                    for half in range(2):
                        nc.sync.dma_start(
                            out=tabsb[:, half * 4096:(half + 1) * 4096]
                            .rearrange("s (b d) -> s b d", d=128),
                            in_=tab[half * 4096:(half + 1) * 4096,
                                    GI[g] * 128:(GI[g] + 1) * 128]
                            .rearrange("(b s) d -> s b d", s=128))
                    for scg in range(8):
                        ct_t = ctp.tile([128, 8 * R], F8, tag="ct",
                                        name=f"ct_{g}_{scg}")
                        nc.sync.dma_start(
                            out=ct_t[:],
                            in_=ct[g][scg * 128:(scg + 1) * 128, :])
                        for sb in range(8):
                            lt = tabsb[:, (scg * 8 + sb) * 128:
                                       (scg * 8 + sb + 1) * 128]
                            for h in range(2):
                                nc.tensor.matmul(
                                    po[h][:],
                                    lhsT=lt,
                                    rhs=ct_t[:, sb * R + h * 512:
                                             sb * R + (h + 1) * 512],
                                    start=first[h], stop=False)
                                first[h] = False
                    for h in range(2):
                        nc.tensor.matmul(po[h][:], lhsT=RW[f"c1_{g}"],
                                         rhs=DEG[g][:, h * 512:(h + 1) * 512],
                                         start=False, stop=False)
                for h in range(2):
                    nc.tensor.matmul(po[h][:], lhsT=W[f"wrT_{t}"],
                                     rhs=ht[t][:, h * 512:(h + 1) * 512],
                                     start=False, stop=False)
                    nc.tensor.matmul(po[h][:], lhsT=RW[f"c0_{t}"],
                                     rhs=ones_row[:], start=False, stop=True)
                    osb = bp.tile([128, 512], F32, tag="osb",
                                  name=f"osb_{t}_{h}")
                    nc.vector.tensor_copy(osb[:], po[h][:])
                    nc.sync.dma_start(out=out_d[t][:, h * 512:(h + 1) * 512],
                                      in_=osb[:])

    nc.compile()
    return nc


# ---------------------------------------------------------------- host prep

def _row_perm():
    """node id -> table row under the half-AllGather layout."""
    n = np.arange(N)
    c = n >> 10
    w = n & 1023
    return (w >> 9) * 4096 + c * 512 + (w & 511)


def _prep(inputs, dbg=False):
    ins = {k: np.asarray(v) for k, v in inputs.items()}

    def bf(x):
        return np.ascontiguousarray(np.asarray(x, np.float32)).astype(BF16)

    com = {}
    for t in "AB":
        iw = ins[f"inW_{t}"].astype(np.float32)
        ib = ins[f"inB_{t}"].astype(np.float32)
        ow = ins[f"outW_{t}"].astype(np.float32)
        ob = ins[f"outB_{t}"].astype(np.float32)
        com[f"wqT_{t}"] = iw[0:128].T
        com[f"wkT_{t}"] = iw[128:256].T
        com[f"wvoT_{t}"] = (ow @ iw[256:384]).T
        com[f"bqs_{t}"] = ib[0:128] * SCALE
        com[f"bk_{t}"] = ib[128:256]
        com[f"bout_eff_{t}"] = ow @ ib[256:384] + ob
    for g in G:
        com[f"wlT_{g}"] = ins[f"wl_{g}"].astype(np.float32).T
        com[f"c1_{g}"] = (ins[f"wl_{g}"].astype(np.float32)
                          @ com[f"bout_eff_{SRC_T[g]}"])
    com["wrT_B"] = ins["wr_AB"].astype(np.float32).T
    com["wrT_A"] = (ins["wr_BA"] + ins["wr_AA"]).astype(np.float32).T
    com["c0_B"] = (ins["bl_AB"].astype(np.float32)
                   + ins["wr_AB"].astype(np.float32) @ com["bout_eff_B"])
    com["c0_A"] = (ins["bl_BA"].astype(np.float32)
                   + ins["bl_AA"].astype(np.float32)
                   + (ins["wr_BA"] + ins["wr_AA"]).astype(np.float32)
                   @ com["bout_eff_A"])

    wblob = bf(np.concatenate([com[k] for k in WB_ORDER], axis=1))
    cblob = np.stack([com[k] for k in CB_ORDER], axis=1).astype(np.float32)

    x0T = {t: np.ascontiguousarray(
        ins[f"x_{t}"][:, 0, :].astype(np.float32).T).astype(BF16)
        for t in "AB"}

    perm = _row_perm()
    cts = {}
    degs = {}
    for g in G:
        src = np.asarray(ins[f"ei_{g}"][0], np.int64)
        dst = np.asarray(ins[f"ei_{g}"][1], np.int64)
        per_core = []
        dgs = []
        for c in range(NCORES):
            sel = (dst >> 10) == c
            s_c = perm[src[sel]]          # permuted table rows
            d_c = dst[sel] - c * R
            cmat = np.zeros((N, R), np.float32)
            np.add.at(cmat, (s_c, d_c), 1.0)
            swz = np.ascontiguousarray(
                cmat.reshape(8, 8, 128, R).transpose(0, 2, 1, 3)
                .reshape(1024, 8 * R))
            per_core.append(swz.astype(FP8))
            dgs.append(np.bincount(d_c, minlength=R).astype(np.float32))
        cts[g] = per_core
        degs[g] = dgs

    in_maps = []
    for c in range(NCORES):
        rblob = np.concatenate(
            [com[k] for k in RB_ORDER] + [degs[g][c] for g in G]
        ).astype(np.float32).reshape(1, -1)
        m = {"wblob": wblob, "cblob": cblob, "rblob": rblob}
        for t in "AB":
            m[f"x0t_{t}"] = x0T[t]
            m[f"x0q_{t}"] = np.ascontiguousarray(x0T[t][:, c * R:(c + 1) * R])
        for g in G:
            m[f"ct_{g}"] = cts[g][c]
        in_maps.append(m)
    return in_maps


def kernel(**inputs):
    in_maps = _prep(inputs)
    if "prog" not in _PROG_CACHE:
        _PROG_CACHE["prog"] = build_program()
    nc = _PROG_CACHE["prog"]
    res = bass_utils.run_bass_kernel_spmd(
        nc, in_maps, core_ids=list(range(NCORES)))
    x_A = np.asarray(inputs["x_A"], np.float32)
    x_B = np.asarray(inputs["x_B"], np.float32)
    new_A = x_A.copy()
    new_B = x_B.copy()
    for c in range(NCORES):
        new_A[c * R:(c + 1) * R, 0, :] = res.results[c]["out_A"].T
        new_B[c * R:(c + 1) * R, 0, :] = res.results[c]["out_B"].T
    return new_A, new_B


# revision 28
# speedup vs baseline: 1.1537x; 1.0642x over previous
"""Trainium2 Bass kernel for nn_AttnDBGNNLayer (8-core SPMD).

kernel(**inputs) takes the FULL inputs (as produced by setup_inputs) and
returns the FULL output (new_A, new_B), distributing across 8 NeuronCores.

Design:
- q-rows of both attentions sharded 8-way (1024 rows/core); K/V computed
  replicated from a feature-major x0^T; A and B q-groups interleaved so the
  TensorEngine always has independent work; single-pass unnormalized softmax
  (scores are tiny; no max subtraction); out-projection folded into V
  (Wvo = Wout @ Wv); softmax row-sum accumulated on DVE+GpSimd, finished
  with a ones-matmul; normalize via PE-transpose + per-partition scale.
- per-graph tables hold h @ wl^T (lin_l folded); the three tables are
  concatenated and AllGathered in two row-halves so the first collective
  hides under the second attention q-group.
- message aggregation as dense count-matrix matmuls: out^T += tab_g^T @ C_g
  with C_g the per-core [8192 src, 1024 dst] edge-count matrix in fp8
  (counts are small ints -> exact); C rows are host-permuted to match the
  AllGather row order and host-swizzled for contiguous streaming.
  lin_r / biases / degree corrections fold in as K=1 matmuls into the same
  PSUM accumulation group.
- outputs are produced feature-major and transposed on the host.
"""
import sys

if "/opt/trn_rl_repo" not in sys.path:
    sys.path.insert(0, "/opt/trn_rl_repo")

import numpy as np
import ml_dtypes

import concourse.bacc as bacc
import concourse.tile as tile
import concourse.mybir as mybir
from concourse import bass_utils

BF16 = ml_dtypes.bfloat16
FP8 = ml_dtypes.float8_e4m3

N = 8192
D = 128
NCORES = 8
R = N // NCORES       # 1024 rows per core
QG = 512              # q-group width
KB = N // 128         # 64 k-blocks
NBLK = R // 128       # 8 dst blocks per core
SCALE = 1.0 / np.sqrt(np.float32(D))

F32 = mybir.dt.float32
BF = mybir.dt.bfloat16
F8 = mybir.dt.float8e4

G = ("AB", "BA", "AA")
GI = {g: i for i, g in enumerate(G)}
SRC_T = {"AB": "A", "BA": "B", "AA": "A"}
GRAPHS_OF = {"A": ("BA", "AA"), "B": ("AB",)}
TABLES_OF = {"A": ("AB", "AA"), "B": ("BA",)}

# bf16 weight blob layout: [128,128] slices
WB_ORDER = ["wqT_A", "wkT_A", "wvoT_A", "wqT_B", "wkT_B", "wvoT_B",
            "wlT_AB", "wlT_BA", "wlT_AA", "wrT_A", "wrT_B"]
# f32 col blob: [128, 4]
CB_ORDER = ["bqs_A", "bk_A", "bqs_B", "bk_B"]
# f32 row blob: [1, 128*5 + 1024*3]
RB_ORDER = ["c0_A", "c0_B", "c1_AB", "c1_BA", "c1_AA"]

_PROG_CACHE = {}


def build_program(dbg=False, stage=3):
    nc = bacc.Bacc("TRN2", target_bir_lowering=False, debug=False,
                   num_devices=NCORES)

    x0t = {t: nc.dram_tensor(f"x0t_{t}", [128, N], BF, kind="ExternalInput")
           for t in "AB"}
    x0q = {t: nc.dram_tensor(f"x0q_{t}", [128, R], BF, kind="ExternalInput")
           for t in "AB"}
    wblob = nc.dram_tensor("wblob", [128, 128 * len(WB_ORDER)], BF,
                           kind="ExternalInput")
    cblob = nc.dram_tensor("cblob", [128, len(CB_ORDER)], F32,
                           kind="ExternalInput")
    rblob = nc.dram_tensor("rblob", [1, 128 * 5 + R * 3], F32,
                           kind="ExternalInput")
    ct = {g: nc.dram_tensor(f"ct_{g}", [1024, 8 * R], F8,
                            kind="ExternalInput") for g in G}
    out_d = {t: nc.dram_tensor(f"out_{t}", [128, R], F32,
                               kind="ExternalOutput") for t in "AB"}
    dbg_d = {}
    if dbg:
        for t in "AB":
            dbg_d[f"ht_{t}"] = nc.dram_tensor(f"dbg_ht_{t}", [128, R], BF,
                                              kind="ExternalOutput")
        dbg_d["tab"] = nc.dram_tensor("dbg_tab", [N, 384], BF,
                                      kind="ExternalOutput")

    tab_loc = nc.dram_tensor("tab_loc", [R, 384], BF)
    tab = nc.dram_tensor("tab", [N, 384], BF, addr_space="Shared")

    with tile.TileContext(nc) as tc:
        with (
            tc.tile_pool(name="const", bufs=1) as cp,
            tc.tile_pool(name="big", bufs=1) as bp,
            tc.tile_pool(name="pt", bufs=4) as ptp,
            tc.tile_pool(name="ctp", bufs=7) as ctp,
            tc.tile_pool(name="ps_s", bufs=2, space="PSUM") as ps_s,
            tc.tile_pool(name="ps_u", bufs=2, space="PSUM") as ps_u,
            tc.tile_pool(name="ps_sm", bufs=2, space="PSUM") as ps_sm,
        ):
            # ---------------- inputs: x0 first, then const blobs
            x0_s = {}
            x0q_s = {}
            for t in "AB":
                x0_s[t] = bp.tile([128, N], BF, tag=f"x0t_{t}",
                                  name=f"x0_{t}")
                nc.sync.dma_start(out=x0_s[t][:], in_=x0t[t][:])
                x0q_s[t] = bp.tile([128, R], BF, tag=f"x0q_{t}",
                                   name=f"x0q_{t}")
                nc.sync.dma_start(out=x0q_s[t][:], in_=x0q[t][:])

            wb = cp.tile([128, 128 * len(WB_ORDER)], BF, tag="wb")
            nc.sync.dma_start(out=wb[:], in_=wblob[:])
            W = {k: wb[:, i * 128:(i + 1) * 128]
                 for i, k in enumerate(WB_ORDER)}
            cb = cp.tile([128, len(CB_ORDER)], F32, tag="cb")
            nc.sync.dma_start(out=cb[:], in_=cblob[:])
            C = {k: cb[:, i:i + 1] for i, k in enumerate(CB_ORDER)}
            rb = cp.tile([1, 128 * 5 + R * 3], F32, tag="rb")
            nc.sync.dma_start(out=rb[:], in_=rblob[:])
            RW = {k: rb[:, i * 128:(i + 1) * 128]
                  for i, k in enumerate(RB_ORDER)}
            DEG = {g: rb[:, 640 + GI[g] * R: 640 + (GI[g] + 1) * R] for g in G}

            ident = cp.tile([128, 128], BF, tag="ident")
            from concourse.masks import make_identity
            make_identity(nc, ident[:])
            ones_col = cp.tile([128, 1], BF, tag="ones_col")
            nc.vector.memset(ones_col[:], 1.0)
            ones_row = cp.tile([1, 512], F32, tag="ones_row")
            nc.vector.memset(ones_row[:], 1.0)

            ht = {t: bp.tile([128, R], BF, tag=f"ht_{t}", name=f"ht_{t}")
                  for t in "AB"}

            # ---------------- QKV for both types
            kt = {}
            vt = {}
            qt = {}
            for t in "AB":
                kt[t] = bp.tile([128, N], BF, tag=f"kt_{t}", name=f"kt_{t}")
                vt[t] = bp.tile([128, N], BF, tag=f"vt_{t}", name=f"vt_{t}")
                qt[t] = bp.tile([128, R], BF, tag=f"qt_{t}", name=f"qt_{t}")
                for j in range(N // 512):
                    ps = ps_s.tile([128, 1024], F32, tag="sc")
                    nc.tensor.matmul(ps[:, :512], lhsT=W[f"wkT_{t}"],
                                     rhs=x0_s[t][:, j * 512:(j + 1) * 512],
                                     start=True, stop=True)
                    nc.scalar.activation(kt[t][:, j * 512:(j + 1) * 512],
                                         ps[:, :512],
                                         mybir.ActivationFunctionType.Identity,
                                         bias=C[f"bk_{t}"])
                for j in range(R // 512):
                    ps = ps_s.tile([128, 1024], F32, tag="sc")
                    nc.tensor.matmul(ps[:, :512], lhsT=W[f"wqT_{t}"],
                                     rhs=x0q_s[t][:, j * 512:(j + 1) * 512],
                                     start=True, stop=True)
                    nc.scalar.activation(qt[t][:, j * 512:(j + 1) * 512],
                                         ps[:, :512],
                                         mybir.ActivationFunctionType.Identity,
                                         bias=C[f"bqs_{t}"],
                                         scale=float(SCALE))
                for vg in range(KB // 4):
                    ps = ps_u.tile([128, 512], F32, tag="ut")
                    for i in range(4):
                        nb = vg * 4 + i
                        nc.tensor.matmul(ps[:, i * 128:(i + 1) * 128],
                                         lhsT=x0_s[t][:, nb * 128:(nb + 1) * 128],
                                         rhs=W[f"wvoT_{t}"],
                                         start=True, stop=True)
                    nc.vector.tensor_copy(vt[t][:, vg * 512:(vg + 1) * 512],
                                          ps[:])

            # ---------------- attention, A/B interleaved; tables per half
            for qg in range(R // QG):
                q_sl = slice(qg * QG, (qg + 1) * QG)
                ut_ps = {}
                racc0 = {}
                racc1 = {}
                for t in "AB":
                    ut_ps[t] = ps_u.tile([128, QG], F32, tag="ut",
                                         name=f"utps_{t}_{qg}")
                    racc0[t] = bp.tile([128, 2 * QG], BF, tag=f"racc0_{t}",
                                       name=f"racc0_{t}_{qg}")
                    racc1[t] = bp.tile([128, 2 * QG], BF, tag=f"racc1_{t}",
                                       name=f"racc1_{t}_{qg}")
                    nc.vector.memset(racc0[t][:], 0.0)
                    nc.gpsimd.memset(racc1[t][:], 0.0)
                for pr in range(KB // 2):
                    kb0 = 2 * pr
                    for t in "AB":
                        sc = ps_s.tile([128, 1024], F32, tag="sc",
                                       name=f"sc_{t}_{pr}")
                        nc.tensor.matmul(sc[:, :512],
                                         lhsT=kt[t][:, kb0 * 128:(kb0 + 1) * 128],
                                         rhs=qt[t][:, q_sl],
                                         start=True, stop=True)
                        nc.tensor.matmul(sc[:, 512:],
                                         lhsT=kt[t][:, (kb0 + 1) * 128:(kb0 + 2) * 128],
                                         rhs=qt[t][:, q_sl],
                                         start=True, stop=True)
                        pt = ptp.tile([128, 1024], BF, tag="pt",
                                      name=f"pt_{t}_{pr}")
                        nc.scalar.activation(pt[:], sc[:],
                                             mybir.ActivationFunctionType.Exp)
                        nc.tensor.matmul(ut_ps[t][:],
                                         lhsT=vt[t][:, kb0 * 128:(kb0 + 1) * 128],
                                         rhs=pt[:, :512],
                                         start=(pr == 0), stop=False)
                        nc.tensor.matmul(ut_ps[t][:],
                                         lhsT=vt[t][:, (kb0 + 1) * 128:(kb0 + 2) * 128],
                                         rhs=pt[:, 512:],
                                         start=False, stop=(pr == KB // 2 - 1))
                        if pr % 4 != 3:
                            nc.vector.tensor_add(racc0[t][:], racc0[t][:],
                                                 pt[:])
                        else:
                            nc.gpsimd.tensor_tensor(racc1[t][:], racc1[t][:],
                                                    pt[:],
                                                    op=mybir.AluOpType.add)

                # normalize + both orientations of h
                for t in "AB":
                    ut_sb = bp.tile([128, QG], BF, tag=f"ut_sb_{t}",
                                    name=f"ut_sb_{t}_{qg}")
                    nc.vector.tensor_copy(ut_sb[:], ut_ps[t][:])
                    for sub in range(QG // 128):
                        s_sl = slice(sub * 128, (sub + 1) * 128)
                        rp = ps_sm.tile([128, 512], F32, tag="sm", name="rp")
                        nc.tensor.matmul(rp[:, :1], lhsT=racc0[t][:, s_sl],
                                         rhs=ones_col[:], start=True,
                                         stop=False)
                        nc.tensor.matmul(rp[:, :1],
                                         lhsT=racc0[t][:, 512 + sub * 128:
                                                      512 + (sub + 1) * 128],
                                         rhs=ones_col[:], start=False,
                                         stop=False)
                        nc.tensor.matmul(rp[:, :1], lhsT=racc1[t][:, s_sl],
                                         rhs=ones_col[:], start=False,
                                         stop=False)
                        nc.tensor.matmul(rp[:, :1],
                                         lhsT=racc1[t][:, 512 + sub * 128:
                                                      512 + (sub + 1) * 128],
                                         rhs=ones_col[:], start=False,
                                         stop=True)
                        rinv = bp.tile([128, 1], F32, tag="rinv")
                        nc.vector.reciprocal(rinv[:], rp[:, :1])
                        tp = ps_sm.tile([128, 512], BF, tag="sm", name="tp")
                        nc.tensor.transpose(tp[:, :128], ut_sb[:, s_sl],
                                            ident[:])
                        hn = bp.tile([128, 128], BF, tag="hn")
                        nc.vector.tensor_scalar_mul(hn[:], tp[:, :128],
                                                    rinv[:, :])
                        tp2 = ps_sm.tile([128, 512], BF, tag="sm", name="tp2")
                        nc.tensor.transpose(tp2[:, :128], hn[:], ident[:])
                        nc.vector.tensor_copy(
                            ht[t][:, qg * QG + sub * 128:
                                  qg * QG + (sub + 1) * 128],
                            tp2[:, :128])

                # tables for this half: local rows qg*512..+512, all graphs
                for t in "AB":
                    for g in TABLES_OF[t]:
                        tsb = bp.tile([128, 4 * 128], BF, tag="tsb",
                                      name=f"tsb_{g}_{qg}")
                        for nb in range(4):
                            wblk = qg * 4 + nb
                            ps = ps_sm.tile([128, 512], F32, tag="sm",
                                            name="tps")
                            nc.tensor.matmul(
                                ps[:, :128],
                                lhsT=ht[t][:, wblk * 128:(wblk + 1) * 128],
                                rhs=W[f"wlT_{g}"], start=True, stop=True)
                            nc.vector.tensor_copy(
                                tsb[:, nb * 128:(nb + 1) * 128], ps[:, :128])
                        for nb in range(4):
                            wblk = qg * 4 + nb
                            nc.sync.dma_start(
                                out=tab_loc[wblk * 128:(wblk + 1) * 128,
                                            GI[g] * 128:(GI[g] + 1) * 128],
                                in_=tsb[:, nb * 128:(nb + 1) * 128])
                # half AllGather: rows [qg*512, qg*512+512) of each core
                nc.gpsimd.collective_compute(
                    "AllGather", mybir.AluOpType.bypass,
                    replica_groups=[list(range(NCORES))],
                    ins=[tab_loc[qg * 512:(qg + 1) * 512, :]],
                    outs=[tab[qg * 4096:(qg + 1) * 4096, :]])

            if dbg:
                for t in "AB":
                    nc.sync.dma_start(out=dbg_d[f"ht_{t}"][:], in_=ht[t][:])
                nc.sync.dma_start(out=dbg_d["tab"][:], in_=tab[:])

            # ---------------- phase 2: dense count-matrix aggregation
            # out^T[d, dst] = sum_g tab_g^T @ C_g + wr@ht + c0*1 + c1*deg
            for t in ("BA" if stage >= 2 else ""):
                po = []
                for h in range(2):
                    po_t = ps_sm.tile([128, 512], F32, tag="sm",
                                      name=f"po_{t}_{h}")
                    po.append(po_t)
                first = [True, True]
                for g in GRAPHS_OF[t]:
                    tabsb = bp.tile([128, KB * 128], BF,
                                    tag=f"x0t_{'A' if GI[g] % 2 == 0 else 'B'}",
                                    name=f"tabsb_{g}")
                    for half in range(2):
                        nc.sync.dma_start(
                            out=tabsb[:, half * 4096:(half + 1) * 4096]
                            .rearrange("s (b d) -> s b d", d=128),
                            in_=tab[half * 4096:(half + 1) * 4096,
                                    GI[g] * 128:(GI[g] + 1) * 128]
                            .rearrange("(b s) d -> s b d", s=128))
                    for scg in range(8):
                        ct_t = ctp.tile([128, 8 * R], F8, tag="ct",
                                        name=f"ct_{g}_{scg}")
                        nc.sync.dma_start(
                            out=ct_t[:],
                            in_=ct[g][scg * 128:(scg + 1) * 128, :])
                        for sb in range(8):
                            lt = tabsb[:, (scg * 8 + sb) * 128:
                                       (scg * 8 + sb + 1) * 128]
                            for h in range(2):
                                nc.tensor.matmul(
                                    po[h][:],
                                    lhsT=lt,
                                    rhs=ct_t[:, sb * R + h * 512:
                                             sb * R + (h + 1) * 512],
                                    start=first[h], stop=False)
                                first[h] = False
                    for h in range(2):
                        nc.tensor.matmul(po[h][:], lhsT=RW[f"c1_{g}"],
                                         rhs=DEG[g][:, h * 512:(h + 1) * 512],
                                         start=False, stop=False)
                for h in range(2):
                    nc.tensor.matmul(po[h][:], lhsT=W[f"wrT_{t}"],
                                     rhs=ht[t][:, h * 512:(h + 1) * 512],
                                     start=False, stop=False)
                    nc.tensor.matmul(po[h][:], lhsT=RW[f"c0_{t}"],
                                     rhs=ones_row[:], start=False, stop=True)
                    osb = bp.tile([128, 512], F32, tag="osb",
                                  name=f"osb_{t}_{h}")
                    nc.vector.tensor_copy(osb[:], po[h][:])
                    nc.sync.dma_start(out=out_d[t][:, h * 512:(h + 1) * 512],
                                      in_=osb[:])

    nc.compile()
    return nc


# ---------------------------------------------------------------- host prep

def _row_perm():
    """node id -> table row under the half-AllGather layout."""
    n = np.arange(N)
    c = n >> 10
    w = n & 1023
    return (w >> 9) * 4096 + c * 512 + (w & 511)


def _prep(inputs, dbg=False):
    ins = {k: np.asarray(v) for k, v in inputs.items()}

    def bf(x):
        return np.ascontiguousarray(np.asarray(x, np.float32)).astype(BF16)

    com = {}
    for t in "AB":
        iw = ins[f"inW_{t}"].astype(np.float32)
        ib = ins[f"inB_{t}"].astype(np.float32)
        ow = ins[f"outW_{t}"].astype(np.float32)
        ob = ins[f"outB_{t}"].astype(np.float32)
        com[f"wqT_{t}"] = iw[0:128].T
        com[f"wkT_{t}"] = iw[128:256].T
        com[f"wvoT_{t}"] = (ow @ iw[256:384]).T
        com[f"bqs_{t}"] = ib[0:128] * SCALE
        com[f"bk_{t}"] = ib[128:256]
        com[f"bout_eff_{t}"] = ow @ ib[256:384] + ob
    for g in G:
        com[f"wlT_{g}"] = ins[f"wl_{g}"].astype(np.float32).T
        com[f"c1_{g}"] = (ins[f"wl_{g}"].astype(np.float32)
                          @ com[f"bout_eff_{SRC_T[g]}"])
    com["wrT_B"] = ins["wr_AB"].astype(np.float32).T
    com["wrT_A"] = (ins["wr_BA"] + ins["wr_AA"]).astype(np.float32).T
    com["c0_B"] = (ins["bl_AB"].astype(np.float32)
                   + ins["wr_AB"].astype(np.float32) @ com["bout_eff_B"])
    com["c0_A"] = (ins["bl_BA"].astype(np.float32)
                   + ins["bl_AA"].astype(np.float32)
                   + (ins["wr_BA"] + ins["wr_AA"]).astype(np.float32)
                   @ com["bout_eff_A"])

    wblob = bf(np.concatenate([com[k] for k in WB_ORDER], axis=1))
    cblob = np.stack([com[k] for k in CB_ORDER], axis=1).astype(np.float32)

    x0T = {t: np.ascontiguousarray(
        ins[f"x_{t}"][:, 0, :].astype(np.float32).T).astype(BF16)
        for t in "AB"}

    perm = _row_perm()
    cts = {}
    degs = {}
    for g in G:
        src = np.asarray(ins[f"ei_{g}"][0], np.int64)
        dst = np.asarray(ins[f"ei_{g}"][1], np.int64)
        per_core = []
        dgs = []
        for c in range(NCORES):
            sel = (dst >> 10) == c
            s_c = perm[src[sel]]          # permuted table rows
            d_c = dst[sel] - c * R
            cmat = np.zeros((N, R), np.float32)
            np.add.at(cmat, (s_c, d_c), 1.0)
            swz = np.ascontiguousarray(
                cmat.reshape(8, 8, 128, R).transpose(0, 2, 1, 3)
                .reshape(1024, 8 * R))
            per_core.append(swz.astype(FP8))
            dgs.append(np.bincount(d_c, minlength=R).astype(np.float32))
        cts[g] = per_core
        degs[g] = dgs

    in_maps = []
    for c in range(NCORES):
        rblob = np.concatenate(
            [com[k] for k in RB_ORDER] + [degs[g][c] for g in G]
        ).astype(np.float32).reshape(1, -1)
        m = {"wblob": wblob, "cblob": cblob, "rblob": rblob}
        for t in "AB":
            m[f"x0t_{t}"] = x0T[t]
            m[f"x0q_{t}"] = np.ascontiguousarray(x0T[t][:, c * R:(c + 1) * R])
        for g in G:
            m[f"ct_{g}"] = cts[g][c]
        in_maps.append(m)
    return in_maps


def kernel(**inputs):
    in_maps = _prep(inputs)
    if "prog" not in _PROG_CACHE:
        _PROG_CACHE["prog"] = build_program()
    nc = _PROG_CACHE["prog"]
    res = bass_utils.run_bass_kernel_spmd(
        nc, in_maps, core_ids=list(range(NCORES)))
    x_A = np.asarray(inputs["x_A"], np.float32)
    x_B = np.asarray(inputs["x_B"], np.float32)
    new_A = x_A.copy()
    new_B = x_B.copy()
    for c in range(NCORES):
        new_A[c * R:(c + 1) * R, 0, :] = res.results[c]["out_A"].T
        new_B[c * R:(c + 1) * R, 0, :] = res.results[c]["out_B"].T
    return new_A, new_B
